# revision 64
# baseline (speedup 1.0000x reference)
"""Trainium2 Bass kernel for nn_DecodingLoss (cepstrum decoding loss).

Math (per 4096-sample window):
  cep = irfft(log(|rfft(x)| + eps))[DELAYS]; softargmax(beta=1e10) ~= hard argmax;
  loss = clip(|idx - symbol|,0,1) = 1[argmax != symbol]; per-audio sums -> 5 scalars.

Kernel strategy (8 cores, pure data parallel over the batch dim; 1024 windows/core):
  FFT 4096 = 32 x 128 Cooley-Tukey, n = 128 t + s (t<32, s<128), k = u + 32 v.
  stage1 (PE): per 4-window group, stationary = x4 [(w4 t), s], moving = block-diag
    W32 table -> psA = A^T[s, (j, w4)] directly (no transpose step). Real-input
    hermitian symmetry: only u=0..16 kept (u0/u16 real), 32 j-cols per window.
    The PSUM->SBUF copy scatters to a j-major `at` so every stage2 moving operand
    is a contiguous 256-column slice (strided movings halve PE stream rate).
  stage2 (PE): for q=1..15 the conjugate k-sets {q+32v} and {32-q+32v} share
    moving operands rre/rim; 128-wide stationaries emit Re of both sets into one
    psX region and Im into another -> |X|^2 = aligned full-width adds. Two q's
    batched per [128,1024] psX ("super-pair") to halve ACT instruction count.
    u=0/16 singleton handled via a PE stacked-identity sum (psM).
  log|X|: ACT Square(scale 2^-6) -> bf16, DVE add, ACT Ln (values centered near 0
  so bf16 is safe), per-pair bf16 projection matmuls accumulate cep[8, 256 win].
  Loss: transpose cep to [win, tap]; batched: sel = cep[sym] via one-hot mult,
  loss = min((max - sel)*1e12, 1). Host sums per-audio errors + final scalars.
  Pipelining: stage1 quads of iteration N+1 are interleaved between the supers of
  iteration N so copies never gate the PE; projections trail their super by 2.
"""
import numpy as np
import ml_dtypes

import concourse.bass as bass
import concourse.mybir as mybir
from concourse import tile
from concourse.bass_utils import run_bass_kernel_spmd

FP32 = mybir.dt.float32
BF16 = mybir.dt.bfloat16
F8 = mybir.dt.float8e4
I32 = mybir.dt.int32
F8NP = ml_dtypes.float8_e4m3fn

B, NW, WIN = 64, 128, 4096
NCORES = 8
BLOC = B // NCORES              # 8 audio rows per core
WLOC = BLOC * NW                # 1024 windows per core
T, S = 32, 128                  # n = 128 t + s
NV = 64                         # v-grid size per k-set
ITERS = 4
WPI = WLOC // ITERS             # 256 windows per iteration
G = WPI // 4                    # 64 groups of 4 windows
DELAYS = np.array([64, 96, 128, 160, 192, 224, 256, 288])
SQ_SCALE = 2.0 ** -6            # |X|^2 scaled by 2^-12: ln output centered near 0
LN_EPS = 2.44e-14

_cache = {}


def _hoist_waits(bir_json):
    """This walrus build rejects instructions carrying attached semaphore waits
    ("Too many sync wait commands"); raw-bass style standalone EventSemaphore
    waits compile and run. Hoist every attached wait into its own
    EventSemaphore on the same engine queue; updates stay attached."""
    import json
    d = json.loads(bir_json)
    n = 0
    for fn in d["functions"]:
        for bb in fn["blocks"]:
            out = []
            for ins in bb["instructions"]:
                si = ins.get("sync_info")
                waits = (si or {}).get("on_wait") or []
                if waits and ins.get("opcode") != "EventSemaphore" and ins.get("engine"):
                    for w in waits:
                        n += 1
                        out.append({
                            "name": f"hoistw-{n}", "opcode": "EventSemaphore",
                            "engine": ins["engine"], "ins": [], "outs": [],
                            "sync_info": {"on_wait": [w], "on_update": []},
                        })
                    si["on_wait"] = []
                out.append(ins)
            bb["instructions"] = out
    return json.dumps(d).encode()


def _install_hoist(nc):
    orig = nc.to_json_bytes
    nc.to_json_bytes = lambda: _hoist_waits(orig())
    return nc


def _tables():
    t = np.arange(T)
    # BDCS [128,128]: rows (w4,t), cols (jj,w4); jj: 0=re u0, 1=re u16,
    # 2..16=re u=1..15, 17..31=im u=1..15
    blk = np.zeros((32, 32))
    blk[:, 0] = 1.0
    blk[:, 1] = np.cos(np.pi * t)
    for u in range(1, 16):
        blk[:, u + 1] = np.cos(2 * np.pi * t * u / 32.0)
        blk[:, u + 16] = -np.sin(2 * np.pi * t * u / 32.0)
    bdcs = np.zeros((128, 128))
    for w in range(4):
        bdcs[w * 32:(w + 1) * 32, w * 32:(w + 1) * 32] = blk
    # j' layout: 0=u0re, 1=u16re, 2q=re(q), 2q+1=im(q) -- re/im adjacent so the
    # stage2 DoubleRow moving operand is one [s, 2, 256] view
    jj_old = [0, 1] + [q + 1 if c == 0 else q + 16
                       for q in range(1, 16) for c in range(2)]
    perm = np.array([w4 * 32 + jj_old[jp] for jp in range(32) for w4 in range(4)])
    bdcs = bdcs[:, perm]


    s = np.arange(S)[:, None]
    v = np.arange(NV)[None, :]
    # ss [128, 62*128]: q=1..15 -> blocks (q-1)*4 + {SR1,SR2,SI1,SI2}; S0=60, S16=61
    ss = np.zeros((128, 62 * 128))
    for q in range(1, 16):
        phA = 2 * np.pi * s * (q + 32 * v) / 4096.0
        phB = 2 * np.pi * s * ((32 - q) + 32 * v) / 4096.0
        o = (q - 1) * 4 * 128
        ss[:, o:o + 128] = np.hstack([np.cos(phA), np.cos(phB)])         # SR1 @ rre
        ss[:, o + 128:o + 256] = np.hstack([np.sin(phA), -np.sin(phB)])   # SR2 @ rim
        ss[:, o + 256:o + 384] = np.hstack([-np.sin(phA), -np.sin(phB)])  # SI1 @ rre
        ss[:, o + 384:o + 512] = np.hstack([np.cos(phA), -np.cos(phB)])   # SI2 @ rim
    ph0 = 2 * np.pi * s * (32 * (v + 1)) / 4096.0
    ph16 = 2 * np.pi * s * (16 + 32 * v) / 4096.0
    ss[:, 60 * 128:61 * 128] = np.hstack([np.cos(ph0), -np.sin(ph0)])
    ss[:, 61 * 128:62 * 128] = np.hstack([np.cos(ph16), -np.sin(ph16)])

    vv = np.arange(NV)

    def ppcol(k):  # [64, 8]
        wk = np.where(k == 2048, 1.0, 2.0)
        return (wk[:, None] * 0.5 *
                np.cos(2 * np.pi * k[:, None] * DELAYS[None, :] / 4096.0) / 4096.0)

    ppj = np.zeros((128, 15 * 8))
    for q in range(1, 16):
        ppj[0:64, (q - 1) * 8:q * 8] = ppcol(q + 32 * vv)
        ppj[64:128, (q - 1) * 8:q * 8] = ppcol((32 - q) + 32 * vv)
    pp016 = np.zeros((64, 16))
    pp016[:, 0:8] = ppcol(32 * (vv + 1))
    pp016[:, 8:16] = ppcol(16 + 32 * vv)

    i64b = np.zeros((128, 64))
    i64b[np.arange(128), np.arange(128) % 64] = 1.0
    ident8 = np.eye(8)
    bf = ml_dtypes.bfloat16
    return (bdcs.astype(F8NP), ss.astype(F8NP), ppj.astype(bf), pp016.astype(bf),
            i64b.astype(bf), ident8.astype(np.float32))


def _build():
    nc = bass.Bass()
    audio = nc.dram_tensor("audio", [ITERS * 128, G * 128], F8, kind="ExternalInput")
    bdcs_d = nc.dram_tensor("bdcs", [128, 128], F8, kind="ExternalInput")
    ss_d = nc.dram_tensor("ss", [128, 62 * 128], F8, kind="ExternalInput")
    ppj_d = nc.dram_tensor("ppj", [128, 120], BF16, kind="ExternalInput")
    pp016_d = nc.dram_tensor("pp016", [64, 16], BF16, kind="ExternalInput")
    oh_d = nc.dram_tensor("onehot", [128, 64], FP32, kind="ExternalInput")
    i64_d = nc.dram_tensor("i64b", [128, 64], BF16, kind="ExternalInput")
    id8_d = nc.dram_tensor("ident8", [8, 8], FP32, kind="ExternalInput")
    loss_out = nc.dram_tensor("loss_out", [128, 8], FP32, kind="ExternalOutput")
    cep_dbg = nc.dram_tensor("cep_dbg", [128, 64], FP32, kind="ExternalOutput")

    with tile.TileContext(nc) as tc:
        with (
            tc.tile_pool(name="consts", bufs=1) as consts,
            tc.tile_pool(name="xt", bufs=2) as xt_pool,
            tc.tile_pool(name="at", bufs=2) as at_pool,
            tc.tile_pool(name="sq", bufs=3) as sq_pool,
            tc.tile_pool(name="m2", bufs=3) as m2_pool,
            tc.tile_pool(name="lg", bufs=6) as lg_pool,
            tc.tile_pool(name="lg0", bufs=2) as lg0_pool,
            tc.tile_pool(name="fin", bufs=2) as fin_pool,
            tc.tile_pool(name="psA", bufs=3, space="PSUM") as psA_pool,
            tc.tile_pool(name="psX", bufs=2, space="PSUM") as psX_pool,
            tc.tile_pool(name="cep", bufs=1, space="PSUM") as cep_pool,
        ):
            # bdcs first (gates the first quad), then block-0 audio (bulk, split
            # over both hwdge rings), then the big ss table on the scalar ring
            bdcs = consts.tile([128, 128], F8, tag="bdcs")
            nc.sync.dma_start(bdcs[:], bdcs_d[:])
            ss = consts.tile([128, 62 * 128], F8, tag="ss")
            ppj = consts.tile([128, 120], BF16, tag="ppj")
            pp016 = consts.tile([64, 16], BF16, tag="pp016")
            onehot = consts.tile([128, 64], FP32, tag="onehot")
            i64b = consts.tile([128, 64], BF16, tag="i64b")
            ident8 = consts.tile([8, 8], FP32, tag="ident8")

            def dma_consts():
                nc.scalar.dma_start(ss[:], ss_d[:])
                nc.scalar.dma_start(ppj[:], ppj_d[:])
                nc.scalar.dma_start(pp016[:], pp016_d[:])
                nc.scalar.dma_start(onehot[:], oh_d[:])
                nc.scalar.dma_start(i64b[:], i64_d[:])
                nc.scalar.dma_start(ident8[:], id8_d[:])

            epsb = consts.tile([128, 1], FP32, tag="epsb")
            nc.vector.memset(epsb[:], LN_EPS)
            cepT = consts.tile([128, 64], FP32, tag="cepT")

            def sblk(b):  # stationary block b of ss
                return ss[:, b * 128:(b + 1) * 128]

            xts = [None, None]
            ats = [None, None]

            # 4 blocks of 256 windows; fp8 DoubleRow stage1 (2 groups/matmul)
            NB = 4

            def dma_xt(blk, eng_split=False):
                xt = xt_pool.tile([128, G * 128], F8, tag="xt")
                for c in range(4):
                    co = c * 2048
                    eng = nc.scalar if (eng_split and c % 2) else nc.sync
                    eng.dma_start(xt[:, co:co + 2048],
                                  audio[blk * 128:(blk + 1) * 128, co:co + 2048])
                xts[blk % 2] = xt
                ats[blk % 2] = at_pool.tile([128, 32 * 256], F8, tag="at", name="at")

            def quad(blk, qd, act_copy=False):
                # stage1: 4 plain fp8 matmuls -> psA [s, (g, j, w4)], then a
                # j-major scatter-copy into `at`
                xt, at = xts[blk % 2], ats[blk % 2]
                g0 = qd * 4
                psA = psA_pool.tile([128, 512], FP32, tag="psA")
                for g in range(g0, g0 + 4):
                    nc.tensor.matmul(psA[:, (g - g0) * 128:(g - g0 + 1) * 128],
                                     xt[:, g * 128:(g + 1) * 128], bdcs[:],
                                     start=True, stop=True)
                atv_j = at[:].rearrange("s (j g w4) -> s j g w4", j=32, w4=4)
                dst = atv_j[:, :, g0:g0 + 4, :]
                src = psA[:].rearrange("s (g j w4) -> s j g w4", g=4, w4=4)
                if act_copy:
                    nc.scalar.activation(dst, src,
                                         mybir.ActivationFunctionType.Copy)
                else:
                    nc.vector.tensor_copy(dst, src)

            class IterCtx:
                pass

            def start_blk(blk):
                ctx = IterCtx()
                ctx.blk = blk
                ctx.at = ats[blk % 2]
                ctx.cep = cep_pool.tile([128, 512], FP32, tag="cep", name="cep")
                ctx.projq = []
                ctx.emitted = 0
                ctx.sq7 = None
                ctx.m2 = None
                return ctx

            def cep_acc(ctx, stat, lgt):
                # accumulating projection matmul; start on first, stop on 17th
                nc.tensor.matmul(ctx.cep[0:8, 0:256], stat, lgt,
                                 start=(ctx.emitted == 0),
                                 stop=(ctx.emitted == 16))
                ctx.emitted += 1

            def add_proj(ctx, stat, lgt):
                ctx.projq.append(lambda ctx=ctx, s=stat, l=lgt: cep_acc(ctx, s, l))

            def emit_proj(ctx):
                ctx.projq.pop(0)()

            def super_pair(ctx, s):
                # two conjugate k-set pairs per psX bank-pair; s=7 packs
                # (q=15, u0/u16)
                at = ctx.at
                psX = psX_pool.tile([128, 1024], FP32, tag="psX")
                DR = mybir.MatmulPerfMode.DoubleRow

                def pair(q, h):
                    # one DoubleRow matmul per region: 2x128-deep contraction
                    # over (s, re/im) with stationary [SR1|SR2] / [SI1|SI2]
                    rr2 = (at[:, 2 * q * 256:(2 * q + 2) * 256]
                           .rearrange("s (two w) -> s two w", two=2))
                    o = (q - 1) * 4
                    wre = (ss[:, o * 128:(o + 2) * 128]
                           .rearrange("s (two f) -> s two f", two=2))
                    wim = (ss[:, (o + 2) * 128:(o + 4) * 128]
                           .rearrange("s (two f) -> s two f", two=2))
                    nc.tensor.matmul(psX[:, h * 512:h * 512 + 256], wre, rr2,
                                     start=True, stop=True, perf_mode=DR)
                    nc.tensor.matmul(psX[:, h * 512 + 256:h * 512 + 512], wim, rr2,
                                     start=True, stop=True, perf_mode=DR)

                if s < 7:
                    qa, qb = 2 * s + 1, 2 * s + 2
                    pair(qa, 0)
                    pair(qb, 1)
                else:
                    pair(15, 0)
                    nc.tensor.matmul(psX[:, 512:768], sblk(60), at[:, 0:256],
                                     start=True, stop=True)
                    nc.tensor.matmul(psX[:, 768:1024], sblk(61), at[:, 256:512],
                                     start=True, stop=True)
                sq = sq_pool.tile([128, 1024], BF16, tag="sq")
                if s in (2, 5):
                    sqc = sq_pool.tile([128, 1024], BF16, tag="sqc", name="sqc")
                    nc.vector.tensor_scalar_mul(sqc[:], psX[:], float(SQ_SCALE))
                    nc.gpsimd.tensor_mul(sq[:], sqc[:], sqc[:])
                else:
                    nc.scalar.activation(sq[:], psX[:],
                                         mybir.ActivationFunctionType.Square,
                                         scale=SQ_SCALE)
                m2 = m2_pool.tile([128, 512], BF16, tag="m2")
                if s < 7:
                    nc.vector.tensor_add(m2[:, 0:256], sq[:, 0:256], sq[:, 256:512])
                    nc.vector.tensor_add(m2[:, 256:512], sq[:, 512:768], sq[:, 768:1024])
                    lg = lg_pool.tile([128, 512], BF16, tag="lg")
                    nc.scalar.activation(lg[:], m2[:],
                                         mybir.ActivationFunctionType.Ln,
                                         bias=epsb[:])
                    add_proj(ctx, ppj[:, (qa - 1) * 8:qa * 8], lg[:, 0:256])
                    add_proj(ctx, ppj[:, (qb - 1) * 8:qb * 8], lg[:, 256:512])
                else:
                    nc.vector.tensor_add(m2[:, 0:256], sq[:, 0:256], sq[:, 256:512])
                    lg = lg_pool.tile([128, 512], BF16, tag="lg")
                    nc.scalar.activation(lg[:, 0:256], m2[:, 0:256],
                                         mybir.ActivationFunctionType.Ln,
                                         bias=epsb[:])
                    add_proj(ctx, ppj[:, 14 * 8:15 * 8], lg[:, 0:256])
                    ctx.sq7 = sq

            def q0_tail(ctx):
                sq = ctx.sq7
                psM = psX_pool.tile([128, 1024], FP32, tag="psX")
                nc.tensor.matmul(psM[0:64, 0:512], i64b[:], sq[:, 512:1024],
                                 start=True, stop=True)
                lg0 = lg0_pool.tile([64, 512], BF16, tag="lg0")
                nc.scalar.activation(lg0[:], psM[0:64, 0:512],
                                     mybir.ActivationFunctionType.Ln,
                                     bias=epsb[0:64])
                cep_acc(ctx, pp016[:, 0:8], lg0[:, 0:256])
                cep_acc(ctx, pp016[:, 8:16], lg0[:, 256:512])

            def fin_tail(ctx):
                cep_sb = fin_pool.tile([8, 256], FP32, tag="cep_sb")
                nc.vector.tensor_copy(cep_sb[:], ctx.cep[0:8, 0:256])
                for c in range(2):
                    gc = ctx.blk * 2 + c
                    psC = ctx.cep[:, 256 + c * 8:256 + (c + 1) * 8]
                    nc.tensor.transpose(psC, cep_sb[:, c * 128:(c + 1) * 128],
                                        ident8[:])
                    nc.vector.tensor_copy(cepT[:, gc * 8:(gc + 1) * 8], psC)

            # ---- prologue: block 0 stage1 (ACT helps with copies; DMA on
            # both hwdge rings since nothing else is running) ----
            dma_xt(0, eng_split=True)
            dma_consts()
            for qd in range(16):
                quad(0, qd)

            prev = None
            for blk in range(NB):
                ctx = start_blk(blk)
                if blk + 1 < NB:
                    dma_xt(blk + 1)
                for s in range(8):
                    super_pair(ctx, s)
                    if blk + 1 < NB:
                        quad(blk + 1, 2 * s)
                        quad(blk + 1, 2 * s + 1)
                    if prev is not None:
                        # previous block's tail, spread across early supers
                        if s == 0:
                            emit_proj(prev)
                            emit_proj(prev)
                            emit_proj(prev)
                        elif s == 1:
                            q0_tail(prev)
                        elif s == 2:
                            fin_tail(prev)
                    if s >= 2:
                        emit_proj(ctx)
                        emit_proj(ctx)
                prev = ctx
            while prev.projq:
                emit_proj(prev)
            q0_tail(prev)
            fin_tail(prev)

            # batched loss over all 1024 windows: [128 w, 8 audios]
            tmp = fin_pool.tile([128, 64], FP32, tag="tmp")
            nc.vector.tensor_mul(tmp[:], cepT[:], onehot[:])
            sel = fin_pool.tile([128, 8], FP32, tag="sel")
            nc.vector.reduce_sum(sel[:], tmp[:].rearrange("p (a j) -> p a j", j=8),
                                 axis=mybir.AxisListType.X)
            mx = fin_pool.tile([128, 8], FP32, tag="mx")
            nc.vector.reduce_max(mx[:], cepT[:].rearrange("p (a j) -> p a j", j=8),
                                 axis=mybir.AxisListType.X)
            df = fin_pool.tile([128, 8], FP32, tag="df")
            nc.vector.tensor_sub(df[:], mx[:], sel[:])
            df2 = fin_pool.tile([128, 8], FP32, tag="df2")
            nc.vector.tensor_scalar_mul(df2[:], df[:], 1e12)
            ls = fin_pool.tile([128, 8], FP32, tag="ls")
            nc.vector.tensor_scalar_min(ls[:], df2[:], 1.0)
            nc.sync.dma_start(loss_out[:], ls[:])
            nc.sync.dma_start(cep_dbg[:], cepT[:])
    return nc


def kernel(audio_batch, symbols_batch, num_errs_no_reverb_batch,
           num_errs_reverb_batch):
    audio_batch = np.asarray(audio_batch)
    symbols_batch = np.asarray(symbols_batch, dtype=np.int32)
    nn_ = np.asarray(num_errs_no_reverb_batch).astype(np.float32)
    nr_ = np.asarray(num_errs_reverb_batch).astype(np.float32)

    if "nc" not in _cache:
        _cache["nc"] = _install_hoist(_build())
        _cache["tabs"] = _tables()
    nc = _cache["nc"]
    bdcs, ss, ppj, pp016, i64b, ident8 = _cache["tabs"]

    # host pre-transpose: [core][it, (w4 t), (g s)] so device DMA is contiguous
    wins = (audio_batch.reshape(NCORES, WLOC, T, S)
            .reshape(NCORES, ITERS, G, 4, T, S)
            .transpose(0, 1, 3, 4, 2, 5)
            .reshape(NCORES, ITERS * 128, G * 128)
            .astype(F8NP))
    sy = symbols_batch.reshape(NCORES, BLOC, NW)
    in_maps = []
    for c in range(NCORES):
        oh = (sy[c].T[:, :, None] == np.arange(8)).astype(np.float32).reshape(128, 64)
        in_maps.append({
            "audio": wins[c], "onehot": oh,
            "bdcs": bdcs, "ss": ss, "ppj": ppj, "pp016": pp016,
            "i64b": i64b, "ident8": ident8,
        })
    import os
    res = run_bass_kernel_spmd(nc, in_maps, core_ids=list(range(NCORES)),
                               trace=bool(os.environ.get("KTRACE")))
    _cache["last_res"] = res
    errs = np.zeros(B, np.float32)
    for c in range(NCORES):
        loss = res.results[c]["loss_out"]          # [128 w, 8 audios]
        errs[c * BLOC:(c + 1) * BLOC] = loss.sum(axis=0, dtype=np.float32)

    tot = np.float32(errs.sum())
    diff = nr_ - nn_
    inv_red = np.where(diff == 0, np.float32(1.0), diff / (nr_ - errs))
    ter = np.float32(inv_red.sum())
    denom = np.float32(B * NW)
    return (np.float32(tot / denom), tot, np.float32(ter / B),
            np.float32(nn_.sum() / denom), np.float32(nr_.sum() / denom))


# revision 65
# speedup vs baseline: 1.2201x; 1.2201x over previous
"""Trainium2 Bass kernel for nn_DecodingLoss (cepstrum decoding loss).

Math (per 4096-sample window):
  cep = irfft(log(|rfft(x)| + eps))[DELAYS]; softargmax(beta=1e10) ~= hard argmax;
  loss = clip(|idx - symbol|,0,1) = 1[argmax != symbol]; per-audio sums -> 5 scalars.

Kernel strategy (8 cores, pure data parallel over the batch dim; 1024 windows/core):
  FFT 4096 = 32 x 128 Cooley-Tukey, n = 128 t + s (t<32, s<128), k = u + 32 v.
  stage1 (PE): per 4-window group, stationary = x4 [(w4 t), s], moving = block-diag
    W32 table -> psA = A^T[s, (j, w4)] directly (no transpose step). Real-input
    hermitian symmetry: only u=0..16 kept (u0/u16 real), 32 j-cols per window.
    The PSUM->SBUF copy scatters to a j-major `at` so every stage2 moving operand
    is a contiguous 256-column slice (strided movings halve PE stream rate).
  stage2 (PE): for q=1..15 the conjugate k-sets {q+32v} and {32-q+32v} share
    moving operands rre/rim; 128-wide stationaries emit Re of both sets into one
    psX region and Im into another -> |X|^2 = aligned full-width adds. Two q's
    batched per [128,1024] psX ("super-pair") to halve ACT instruction count.
    u=0/16 singleton handled via a PE stacked-identity sum (psM).
  log|X|: ACT Square(scale 2^-6) -> bf16, DVE add, ACT Ln (values centered near 0
  so bf16 is safe), per-pair bf16 projection matmuls accumulate cep[8, 256 win].
  Loss: transpose cep to [win, tap]; batched: sel = cep[sym] via one-hot mult,
  loss = min((max - sel)*1e12, 1). Host sums per-audio errors + final scalars.
  Pipelining: stage1 quads of iteration N+1 are interleaved between the supers of
  iteration N so copies never gate the PE; projections trail their super by 2.
"""
import numpy as np
import ml_dtypes

import concourse.bass as bass
import concourse.mybir as mybir
from concourse import tile
from concourse.bass_utils import run_bass_kernel_spmd

FP32 = mybir.dt.float32
BF16 = mybir.dt.bfloat16
F8 = mybir.dt.float8e4
I32 = mybir.dt.int32
F8NP = ml_dtypes.float8_e4m3fn

B, NW, WIN = 64, 128, 4096
NCORES = 8
BLOC = B // NCORES              # 8 audio rows per core
WLOC = BLOC * NW                # 1024 windows per core
T, S = 32, 128                  # n = 128 t + s
NV = 64                         # v-grid size per k-set
ITERS = 4
WPI = WLOC // ITERS             # 256 windows per iteration
G = WPI // 4                    # 64 groups of 4 windows
DELAYS = np.array([64, 96, 128, 160, 192, 224, 256, 288])
SQ_SCALE = 2.0 ** -6            # |X|^2 scaled by 2^-12: ln output centered near 0
LN_EPS = 2.44e-14

_cache = {}


def _hoist_waits(bir_json):
    """This walrus build rejects instructions carrying attached semaphore waits
    ("Too many sync wait commands"); raw-bass style standalone EventSemaphore
    waits compile and run. Hoist every attached wait into its own
    EventSemaphore on the same engine queue; updates stay attached."""
    import json
    d = json.loads(bir_json)
    n = 0
    for fn in d["functions"]:
        for bb in fn["blocks"]:
            out = []
            for ins in bb["instructions"]:
                si = ins.get("sync_info")
                waits = (si or {}).get("on_wait") or []
                if waits and ins.get("opcode") != "EventSemaphore" and ins.get("engine"):
                    for w in waits:
                        n += 1
                        out.append({
                            "name": f"hoistw-{n}", "opcode": "EventSemaphore",
                            "engine": ins["engine"], "ins": [], "outs": [],
                            "sync_info": {"on_wait": [w], "on_update": []},
                        })
                    si["on_wait"] = []
                out.append(ins)
            bb["instructions"] = out
    return json.dumps(d).encode()


def _install_hoist(nc):
    orig = nc.to_json_bytes
    nc.to_json_bytes = lambda: _hoist_waits(orig())
    return nc


def _tables():
    t = np.arange(T)
    # BDCS [128,128]: rows (w4,t), cols (jj,w4); jj: 0=re u0, 1=re u16,
    # 2..16=re u=1..15, 17..31=im u=1..15
    blk = np.zeros((32, 32))
    blk[:, 0] = 1.0
    blk[:, 1] = np.cos(np.pi * t)
    for u in range(1, 16):
        blk[:, u + 1] = np.cos(2 * np.pi * t * u / 32.0)
        blk[:, u + 16] = -np.sin(2 * np.pi * t * u / 32.0)
    bdcs = np.zeros((128, 128))
    for w in range(4):
        bdcs[w * 32:(w + 1) * 32, w * 32:(w + 1) * 32] = blk
    # j' layout: 0=u0re, 1=u16re, 2q=re(q), 2q+1=im(q) -- re/im adjacent so the
    # stage2 DoubleRow moving operand is one [s, 2, 256] view
    jj_old = [0, 1] + [q + 1 if c == 0 else q + 16
                       for q in range(1, 16) for c in range(2)]
    perm = np.array([w4 * 32 + jj_old[jp] for jp in range(32) for w4 in range(4)])
    bdcs = bdcs[:, perm]


    s = np.arange(S)[:, None]
    v = np.arange(NV)[None, :]
    # ss [128, 62*128]: q=1..15 -> blocks (q-1)*4 + {SR1,SR2,SI1,SI2}; S0=60, S16=61
    ss = np.zeros((128, 62 * 128))
    for q in range(1, 16):
        phA = 2 * np.pi * s * (q + 32 * v) / 4096.0
        phB = 2 * np.pi * s * ((32 - q) + 32 * v) / 4096.0
        o = (q - 1) * 4 * 128
        ss[:, o:o + 128] = np.hstack([np.cos(phA), np.cos(phB)])         # SR1 @ rre
        ss[:, o + 128:o + 256] = np.hstack([np.sin(phA), -np.sin(phB)])   # SR2 @ rim
        ss[:, o + 256:o + 384] = np.hstack([-np.sin(phA), -np.sin(phB)])  # SI1 @ rre
        ss[:, o + 384:o + 512] = np.hstack([np.cos(phA), -np.cos(phB)])   # SI2 @ rim
    ph0 = 2 * np.pi * s * (32 * (v + 1)) / 4096.0
    ph16 = 2 * np.pi * s * (16 + 32 * v) / 4096.0
    ss[:, 60 * 128:61 * 128] = np.hstack([np.cos(ph0), -np.sin(ph0)])
    ss[:, 61 * 128:62 * 128] = np.hstack([np.cos(ph16), -np.sin(ph16)])

    vv = np.arange(NV)

    def ppcol(k):  # [64, 8]
        wk = np.where(k == 2048, 1.0, 2.0)
        return (wk[:, None] * 0.5 *
                np.cos(2 * np.pi * k[:, None] * DELAYS[None, :] / 4096.0) / 4096.0)

    ppj = np.zeros((128, 15 * 8))
    for q in range(1, 16):
        ppj[0:64, (q - 1) * 8:q * 8] = ppcol(q + 32 * vv)
        ppj[64:128, (q - 1) * 8:q * 8] = ppcol((32 - q) + 32 * vv)
    pp016 = np.zeros((64, 16))
    pp016[:, 0:8] = ppcol(32 * (vv + 1))
    pp016[:, 8:16] = ppcol(16 + 32 * vv)

    i64b = np.zeros((128, 64))
    i64b[np.arange(128), np.arange(128) % 64] = 1.0
    ident8 = np.eye(8)
    bf = ml_dtypes.bfloat16
    return (bdcs.astype(F8NP), ss.astype(F8NP), ppj.astype(bf), pp016.astype(bf),
            i64b.astype(bf), ident8.astype(np.float32))


def _build():
    nc = bass.Bass()
    audio = nc.dram_tensor("audio", [ITERS * 128, G * 128], F8, kind="ExternalInput")
    bdcs_d = nc.dram_tensor("bdcs", [128, 128], F8, kind="ExternalInput")
    ss_d = nc.dram_tensor("ss", [128, 62 * 128], F8, kind="ExternalInput")
    ppj_d = nc.dram_tensor("ppj", [128, 120], BF16, kind="ExternalInput")
    pp016_d = nc.dram_tensor("pp016", [64, 16], BF16, kind="ExternalInput")
    oh_d = nc.dram_tensor("onehot", [128, 64], FP32, kind="ExternalInput")
    i64_d = nc.dram_tensor("i64b", [128, 64], BF16, kind="ExternalInput")
    id8_d = nc.dram_tensor("ident8", [8, 8], FP32, kind="ExternalInput")
    loss_out = nc.dram_tensor("loss_out", [128, 8], FP32, kind="ExternalOutput")
    cep_dbg = nc.dram_tensor("cep_dbg", [128, 64], FP32, kind="ExternalOutput")

    with tile.TileContext(nc) as tc:
        with (
            tc.tile_pool(name="consts", bufs=1) as consts,
            tc.tile_pool(name="xt", bufs=2) as xt_pool,
            tc.tile_pool(name="at", bufs=2) as at_pool,
            tc.tile_pool(name="sq", bufs=3) as sq_pool,
            tc.tile_pool(name="m2", bufs=3) as m2_pool,
            tc.tile_pool(name="lg", bufs=6) as lg_pool,
            tc.tile_pool(name="lg0", bufs=2) as lg0_pool,
            tc.tile_pool(name="fin", bufs=2) as fin_pool,
            tc.tile_pool(name="psA", bufs=3, space="PSUM") as psA_pool,
            tc.tile_pool(name="psX", bufs=2, space="PSUM") as psX_pool,
            tc.tile_pool(name="cep", bufs=1, space="PSUM") as cep_pool,
        ):
            # bdcs first (gates the first quad), then block-0 audio (bulk, split
            # over both hwdge rings), then the big ss table on the scalar ring
            bdcs = consts.tile([128, 128], F8, tag="bdcs")
            nc.sync.dma_start(bdcs[:], bdcs_d[:])
            ss = consts.tile([128, 62 * 128], F8, tag="ss")
            ppj = consts.tile([128, 120], BF16, tag="ppj")
            pp016 = consts.tile([64, 16], BF16, tag="pp016")
            onehot = consts.tile([128, 64], FP32, tag="onehot")
            i64b = consts.tile([128, 64], BF16, tag="i64b")
            ident8 = consts.tile([8, 8], FP32, tag="ident8")

            def dma_consts():
                nc.scalar.dma_start(ss[:], ss_d[:])
                nc.scalar.dma_start(ppj[:], ppj_d[:])
                nc.scalar.dma_start(pp016[:], pp016_d[:])
                nc.scalar.dma_start(onehot[:], oh_d[:])
                nc.scalar.dma_start(i64b[:], i64_d[:])
                nc.scalar.dma_start(ident8[:], id8_d[:])

            epsb = consts.tile([128, 1], FP32, tag="epsb")
            nc.vector.memset(epsb[:], LN_EPS)
            cepT = consts.tile([128, 64], FP32, tag="cepT")

            def sblk(b):  # stationary block b of ss
                return ss[:, b * 128:(b + 1) * 128]

            xts = [None, None]
            ats = [None, None]

            # 4 blocks of 256 windows; fp8 DoubleRow stage1 (2 groups/matmul)
            NB = 4

            def dma_xt(blk, eng_split=False):
                xt = xt_pool.tile([128, G * 128], F8, tag="xt")
                for c in range(4):
                    co = c * 2048
                    eng = nc.scalar if (eng_split and c % 2) else nc.sync
                    eng.dma_start(xt[:, co:co + 2048],
                                  audio[blk * 128:(blk + 1) * 128, co:co + 2048])
                xts[blk % 2] = xt
                ats[blk % 2] = at_pool.tile([128, 32 * 256], F8, tag="at", name="at")

            def quad(blk, qd, act_copy=False):
                # stage1: 4 plain fp8 matmuls -> psA [s, (g, j, w4)], then a
                # j-major scatter-copy into `at`
                xt, at = xts[blk % 2], ats[blk % 2]
                g0 = qd * 4
                psA = psA_pool.tile([128, 512], FP32, tag="psA")
                for g in range(g0, g0 + 4):
                    nc.tensor.matmul(psA[:, (g - g0) * 128:(g - g0 + 1) * 128],
                                     xt[:, g * 128:(g + 1) * 128], bdcs[:],
                                     start=True, stop=True)
                atv_j = at[:].rearrange("s (j g w4) -> s j g w4", j=32, w4=4)
                dst = atv_j[:, :, g0:g0 + 4, :]
                src = psA[:].rearrange("s (g j w4) -> s j g w4", g=4, w4=4)
                if act_copy:
                    nc.scalar.activation(dst, src,
                                         mybir.ActivationFunctionType.Copy)
                else:
                    nc.vector.tensor_copy(dst, src)

            class IterCtx:
                pass

            def start_blk(blk):
                ctx = IterCtx()
                ctx.blk = blk
                ctx.at = ats[blk % 2]
                ctx.cep = cep_pool.tile([128, 512], FP32, tag="cep", name="cep")
                ctx.projq = []
                ctx.emitted = 0
                ctx.sq7 = None
                ctx.m2 = None
                return ctx

            def cep_acc(ctx, stat, lgt):
                # accumulating projection matmul; start on first, stop on 17th
                nc.tensor.matmul(ctx.cep[0:8, 0:256], stat, lgt,
                                 start=(ctx.emitted == 0),
                                 stop=(ctx.emitted == 16))
                ctx.emitted += 1

            def add_proj(ctx, stat, lgt):
                ctx.projq.append(lambda ctx=ctx, s=stat, l=lgt: cep_acc(ctx, s, l))

            def emit_proj(ctx):
                ctx.projq.pop(0)()

            def super_pair(ctx, s):
                # two conjugate k-set pairs per psX bank-pair; s=7 packs
                # (q=15, u0/u16)
                at = ctx.at
                psX = psX_pool.tile([128, 1024], FP32, tag="psX")
                DR = mybir.MatmulPerfMode.DoubleRow

                def pair(q, h):
                    # one DoubleRow matmul per region: 2x128-deep contraction
                    # over (s, re/im) with stationary [SR1|SR2] / [SI1|SI2]
                    rr2 = (at[:, 2 * q * 256:(2 * q + 2) * 256]
                           .rearrange("s (two w) -> s two w", two=2))
                    o = (q - 1) * 4
                    wre = (ss[:, o * 128:(o + 2) * 128]
                           .rearrange("s (two f) -> s two f", two=2))
                    wim = (ss[:, (o + 2) * 128:(o + 4) * 128]
                           .rearrange("s (two f) -> s two f", two=2))
                    nc.tensor.matmul(psX[:, h * 512:h * 512 + 256], wre, rr2,
                                     start=True, stop=True, perf_mode=DR)
                    nc.tensor.matmul(psX[:, h * 512 + 256:h * 512 + 512], wim, rr2,
                                     start=True, stop=True, perf_mode=DR)

                if s < 7:
                    qa, qb = 2 * s + 1, 2 * s + 2
                    pair(qa, 0)
                    pair(qb, 1)
                else:
                    pair(15, 0)
                    nc.tensor.matmul(psX[:, 512:768], sblk(60), at[:, 0:256],
                                     start=True, stop=True)
                    nc.tensor.matmul(psX[:, 768:1024], sblk(61), at[:, 256:512],
                                     start=True, stop=True)
                sq = sq_pool.tile([128, 1024], BF16, tag="sq")
                nc.scalar.activation(sq[:], psX[:],
                                     mybir.ActivationFunctionType.Square,
                                     scale=SQ_SCALE)
                m2 = m2_pool.tile([128, 512], BF16, tag="m2")
                if s < 7:
                    nc.vector.tensor_add(m2[:, 0:256], sq[:, 0:256], sq[:, 256:512])
                    nc.vector.tensor_add(m2[:, 256:512], sq[:, 512:768], sq[:, 768:1024])
                    lg = lg_pool.tile([128, 512], BF16, tag="lg")
                    nc.scalar.activation(lg[:], m2[:],
                                         mybir.ActivationFunctionType.Ln,
                                         bias=epsb[:])
                    add_proj(ctx, ppj[:, (qa - 1) * 8:qa * 8], lg[:, 0:256])
                    add_proj(ctx, ppj[:, (qb - 1) * 8:qb * 8], lg[:, 256:512])
                else:
                    nc.vector.tensor_add(m2[:, 0:256], sq[:, 0:256], sq[:, 256:512])
                    lg = lg_pool.tile([128, 512], BF16, tag="lg")
                    nc.scalar.activation(lg[:, 0:256], m2[:, 0:256],
                                         mybir.ActivationFunctionType.Ln,
                                         bias=epsb[:])
                    add_proj(ctx, ppj[:, 14 * 8:15 * 8], lg[:, 0:256])
                    ctx.sq7 = sq

            def q0_tail(ctx):
                sq = ctx.sq7
                psM = psX_pool.tile([128, 1024], FP32, tag="psX")
                nc.tensor.matmul(psM[0:64, 0:512], i64b[:], sq[:, 512:1024],
                                 start=True, stop=True)
                lg0 = lg0_pool.tile([64, 512], BF16, tag="lg0")
                nc.scalar.activation(lg0[:], psM[0:64, 0:512],
                                     mybir.ActivationFunctionType.Ln,
                                     bias=epsb[0:64])
                cep_acc(ctx, pp016[:, 0:8], lg0[:, 0:256])
                cep_acc(ctx, pp016[:, 8:16], lg0[:, 256:512])

            def fin_tail(ctx):
                cep_sb = fin_pool.tile([8, 256], FP32, tag="cep_sb")
                nc.vector.tensor_copy(cep_sb[:], ctx.cep[0:8, 0:256])
                for c in range(2):
                    gc = ctx.blk * 2 + c
                    psC = ctx.cep[:, 256 + c * 8:256 + (c + 1) * 8]
                    nc.tensor.transpose(psC, cep_sb[:, c * 128:(c + 1) * 128],
                                        ident8[:])
                    nc.vector.tensor_copy(cepT[:, gc * 8:(gc + 1) * 8], psC)

            # ---- prologue: block 0 stage1 (ACT helps with copies; DMA on
            # both hwdge rings since nothing else is running) ----
            dma_xt(0, eng_split=True)
            dma_consts()
            for qd in range(16):
                quad(0, qd)

            prev = None
            for blk in range(NB):
                ctx = start_blk(blk)
                if blk + 1 < NB:
                    dma_xt(blk + 1)
                for s in range(8):
                    super_pair(ctx, s)
                    if blk + 1 < NB:
                        quad(blk + 1, 2 * s)
                        quad(blk + 1, 2 * s + 1)
                    if prev is not None:
                        # previous block's tail, spread across early supers
                        if s == 0:
                            emit_proj(prev)
                            emit_proj(prev)
                            emit_proj(prev)
                        elif s == 1:
                            q0_tail(prev)
                        elif s == 2:
                            fin_tail(prev)
                    if s >= 2:
                        emit_proj(ctx)
                        emit_proj(ctx)
                prev = ctx
            while prev.projq:
                emit_proj(prev)
            q0_tail(prev)
            fin_tail(prev)

            # batched loss over all 1024 windows: [128 w, 8 audios]
            tmp = fin_pool.tile([128, 64], FP32, tag="tmp")
            nc.vector.tensor_mul(tmp[:], cepT[:], onehot[:])
            sel = fin_pool.tile([128, 8], FP32, tag="sel")
            nc.vector.reduce_sum(sel[:], tmp[:].rearrange("p (a j) -> p a j", j=8),
                                 axis=mybir.AxisListType.X)
            mx = fin_pool.tile([128, 8], FP32, tag="mx")
            nc.vector.reduce_max(mx[:], cepT[:].rearrange("p (a j) -> p a j", j=8),
                                 axis=mybir.AxisListType.X)
            df = fin_pool.tile([128, 8], FP32, tag="df")
            nc.vector.tensor_sub(df[:], mx[:], sel[:])
            df2 = fin_pool.tile([128, 8], FP32, tag="df2")
            nc.vector.tensor_scalar_mul(df2[:], df[:], 1e12)
            ls = fin_pool.tile([128, 8], FP32, tag="ls")
            nc.vector.tensor_scalar_min(ls[:], df2[:], 1.0)
            nc.sync.dma_start(loss_out[:], ls[:])
            nc.sync.dma_start(cep_dbg[:], cepT[:])
    return nc


def kernel(audio_batch, symbols_batch, num_errs_no_reverb_batch,
           num_errs_reverb_batch):
    audio_batch = np.asarray(audio_batch)
    symbols_batch = np.asarray(symbols_batch, dtype=np.int32)
    nn_ = np.asarray(num_errs_no_reverb_batch).astype(np.float32)
    nr_ = np.asarray(num_errs_reverb_batch).astype(np.float32)

    if "nc" not in _cache:
        _cache["nc"] = _install_hoist(_build())
        _cache["tabs"] = _tables()
    nc = _cache["nc"]
    bdcs, ss, ppj, pp016, i64b, ident8 = _cache["tabs"]

    # host pre-transpose: [core][it, (w4 t), (g s)] so device DMA is contiguous
    wins = (audio_batch.reshape(NCORES, WLOC, T, S)
            .reshape(NCORES, ITERS, G, 4, T, S)
            .transpose(0, 1, 3, 4, 2, 5)
            .reshape(NCORES, ITERS * 128, G * 128)
            .astype(F8NP))
    sy = symbols_batch.reshape(NCORES, BLOC, NW)
    in_maps = []
    for c in range(NCORES):
        oh = (sy[c].T[:, :, None] == np.arange(8)).astype(np.float32).reshape(128, 64)
        in_maps.append({
            "audio": wins[c], "onehot": oh,
            "bdcs": bdcs, "ss": ss, "ppj": ppj, "pp016": pp016,
            "i64b": i64b, "ident8": ident8,
        })
    import os
    res = run_bass_kernel_spmd(nc, in_maps, core_ids=list(range(NCORES)),
                               trace=bool(os.environ.get("KTRACE")))
    _cache["last_res"] = res
    errs = np.zeros(B, np.float32)
    for c in range(NCORES):
        loss = res.results[c]["loss_out"]          # [128 w, 8 audios]
        errs[c * BLOC:(c + 1) * BLOC] = loss.sum(axis=0, dtype=np.float32)

    tot = np.float32(errs.sum())
    diff = nr_ - nn_
    inv_red = np.where(diff == 0, np.float32(1.0), diff / (nr_ - errs))
    ter = np.float32(inv_red.sum())
    denom = np.float32(B * NW)
    return (np.float32(tot / denom), tot, np.float32(ter / B),
            np.float32(nn_.sum() / denom), np.float32(nr_.sum() / denom))
